# revision 1
# baseline (speedup 1.0000x reference)
"""Trainium2 Bass kernel for nn_AddDropMRR (add-drop microring resonator).

Math: rotate the complex plane per wavelength by -arg(G) (magnitudes are
invariant), where G = t2*s1/den is the ring response. With u = P*x + s*a
and v = (r+P)*x + s*a:

  through = sqrt(g^2*v^2 + c2^2*x^2)        [all coefs per-wavelength]
  drop    = k2c * sqrt(u^2 + Q^2*x^2)

All per-wavelength coefficients depend only on `wavelengths` (8192 values)
and scalar params -> computed on HOST, DMA'd as small f32/f16 tables. The
device graph is pure streaming, software-pipelined in 5 skewed stages so no
engine queue blocks on a same-round cross-engine dependency:

  s0 DMA(qSP):  load x, a' chunk                      (~3.5us/chunk)
  s1 DVE:       u, v via TS-ptr (4x fp16) + TT (2x)   (~4.1us)
  s2 DVE:       u^2, v^2, x^2 in place                (~3.6us)
  s3 PE:        W2 = diag(g^2)@vv + diag(c2^2)@xx,
                D2 = diag(Q^2)@xx + I@uu  -> PSUM     (~5us, 16 matmuls)
  s4 ACT:       4x Sqrt([128,1024] PSUM half) -> fp16, output DMAs (qAct)

GPSIMD is deliberately idle: its SBUF port is shared with DVE's 2-port
perf modes, so gpsimd work serializes against the 4x TS ops (measured).
Tensors ride fp16 (2-byte dtype enables the DVE fast modes; better
mantissa than bf16). PSUM accumulates in f32 so no overflow rescaling is
needed. Sharding: wavelength dim split 8 ways across cores (data-parallel,
fully elementwise); host transposes so wavelength lies on SBUF partitions.
"""
import numpy as np

B = 2048           # batch
W = 8192           # wavelengths
NCORES = 8
WSH = W // NCORES  # 1024 wavelengths per core
P = 128            # SBUF partitions
NCHUNK = WSH // P  # 8 chunks per core
NCOEF = 5          # P, r+P, g/32, (c2/32)^2, Q^2
N_EFF = 2.4
CIRC = 2.0 * np.pi * 1e-05
MODE = "b"         # 'b': W2/D2 on PE;  'v': v on PE, adds on DVE


def _host_prep(wavelengths, coupling_1, coupling_2, phi_1, phi_2, phi_ring,
               alpha):
    """Scalars + per-wavelength coefficient vectors (f64 -> f32)."""
    c1 = float(np.asarray(coupling_1).reshape(-1)[0])
    c2 = float(np.asarray(coupling_2).reshape(-1)[0])
    p1 = float(np.asarray(phi_1).reshape(-1)[0])
    pr = float(np.asarray(phi_ring).reshape(-1)[0])
    al = float(np.asarray(alpha).reshape(-1)[0])
    k1c = float(np.clip(c1, 0.01, 0.99))
    k2c = float(np.clip(c2, 0.01, 0.99))
    t1 = float(np.sqrt(1.0 - k1c * k1c))
    t2 = float(np.sqrt(1.0 - k2c * k2c))
    s = float(np.sqrt(c2))       # unclamped, as in reference
    s1 = float(np.sqrt(c1))      # unclamped
    kappa = float(al * np.sqrt(1.0 - c1 * c1) * np.sqrt(1.0 - c2 * c2))

    # phi in f32 exactly as the reference computes it, then f64 trig
    wl = np.asarray(wavelengths, np.float32)
    phi32 = (np.float32(2.0 * np.pi * N_EFF) / wl) * np.float32(CIRC) \
        + np.float32(pr)
    phi = phi32.astype(np.float64)
    sin_p = np.sin(phi + p1)
    cos_p = np.cos(phi + p1)
    sin_f = np.sin(phi)
    cos_f = np.cos(phi)

    Pv = -k1c * al * sin_p
    Qv = k1c * al * cos_p
    den_re = 1.0 - kappa * cos_f
    den2 = den_re * den_re + (kappa * sin_f) ** 2
    rsq = 1.0 / np.sqrt(den2)
    g = (t2 * s1) * rsq
    r = (t1 / (t2 * s1)) * den_re
    c2v = (t2 * s1 * Qv - t1 * kappa * sin_f) * rsq

    coefs = np.stack([
        Pv,
        r + Pv,
        g / 32.0,
        (c2v / 32.0) ** 2,
        Qv ** 2,
    ]).astype(np.float32)                       # [NCOEF, W]
    vecs = dict(g2=(g * g), c22=(c2v * c2v), q2=(Qv * Qv), rp=(r + Pv))
    return coefs, dict(s=s, k2c=k2c, **{k: v.astype(np.float32)
                                        for k, v in vecs.items()})


def _build_graph(k2c, loop_n=1, nchunk=NCHUNK, bufs=10, mode=MODE,
                 split_dma=True, passes=1, taper=1, swdge_o2=True, xx_gp=False,
                 merged=False):
    """SPMD per-core graph; see module docstring. loop_n>1 wraps the body
    in an on-device For_i loop for steady-state timing."""
    import concourse.tile as tile
    from concourse import bacc, mybir, bass

    f32 = mybir.dt.float32
    f16 = mybir.dt.float16
    AF = mybir.ActivationFunctionType
    ALU = mybir.AluOpType

    wsh = nchunk * P
    ndiag = (3 * nchunk + 1) if mode == "b" else (nchunk + 1)
    nc = bacc.Bacc("TRN2", target_bir_lowering=False, debug=False,
                   num_devices=NCORES)
    x_ext = nc.declare_dram_parameter("x_t", [wsh, B], f16, isOutput=False)
    a_ext = nc.declare_dram_parameter("a_t", [wsh, B], f16, isOutput=False)
    cf_ext = nc.declare_dram_parameter("cf_t", [P, NCOEF * nchunk], f32,
                                       isOutput=False)
    dg_ext = nc.declare_dram_parameter("dg_t", [P, ndiag * P], f16,
                                       isOutput=False)
    o1_ext = nc.declare_dram_parameter("o1_t", [wsh, B], f16, isOutput=True)
    o2_ext = nc.declare_dram_parameter("o2_t", [wsh, B], f16, isOutput=True)

    k2sq = float(k2c * k2c)
    out_eng = "scalar" if split_dma else "sync"
    HB = B // 2  # psum half width

    with tile.TileContext(nc) as tc:
        with tc.tile_pool(name="cst", bufs=1) as cst, \
             tc.tile_pool(name="mio", bufs=bufs) as mio, \
             tc.tile_pool(name="psum", bufs=2,
                          space=bass.MemorySpace.PSUM) as psum:

            def body(_iv=None):
                cf = cst.tile([P, NCOEF * nchunk], f32, tag="cf", name="cf")
                nc.sync.dma_start(cf[:], cf_ext[:])
                dg = cst.tile([P, ndiag * P], f16, tag="dg", name="dg")
                nc.sync.dma_start(dg[:], dg_ext[:])

                def C(k, c):
                    return cf[:, k * nchunk + c:k * nchunk + c + 1]

                def DG(k):  # k-th [P,P] diag block
                    return dg[:, k * P:(k + 1) * P]

                st = {}

                def s0(vc):
                    v_id, c, lo, hi = vc
                    rs = slice(c * P, (c + 1) * P)
                    cs = slice(lo, hi)
                    if merged:
                        # packed [x | u | v] tile: one wide TT for both adds
                        # and one for all three squares
                        xuv = mio.tile([P, 3, B], f16, tag="xuv", name="xuv")
                        nc.sync.dma_start(xuv[:, 0, cs], x_ext[rs, cs])
                        at = mio.tile([P, B], f16, tag="at", name="at")
                        nc.sync.dma_start(at[:, cs], a_ext[rs, cs])
                        st[v_id] = dict(xuv=xuv, at=at)
                        return
                    xt = mio.tile([P, B], f16, tag="xt", name="xt")
                    nc.sync.dma_start(xt[:, cs], x_ext[rs, cs])
                    at = mio.tile([P, B], f16, tag="at", name="at")
                    nc.sync.dma_start(at[:, cs], a_ext[rs, cs])
                    st[v_id] = dict(xt=xt, at=at)

                def s1(vc):
                    v_id, c, lo, hi = vc
                    cs = slice(lo, hi)
                    d = st[v_id]
                    if merged:
                        xuv, at = d["xuv"], d["at"]
                        w = hi - lo
                        nc.vector.tensor_scalar(xuv[:, 1, cs], xuv[:, 0, cs],
                                                C(0, c), None, ALU.mult)
                        nc.vector.tensor_scalar(xuv[:, 2, cs], xuv[:, 0, cs],
                                                C(1, c), None, ALU.mult)
                        arep = at[:, None, cs].broadcast_to([P, 2, w])
                        nc.vector.tensor_add(xuv[:, 1:3, cs],
                                             xuv[:, 1:3, cs], arep)
                        nc.vector.tensor_mul(xuv[:, :, cs], xuv[:, :, cs],
                                             xuv[:, :, cs])
                        return
                    xt, at = d["xt"], d["at"]
                    ut = mio.tile([P, B], f16, tag="ut", name="ut")
                    nc.vector.tensor_scalar(ut[:, cs], xt[:, cs], C(0, c),
                                            None, ALU.mult)
                    nc.vector.tensor_add(ut[:, cs], ut[:, cs], at[:, cs])
                    d["ut"] = ut
                    if mode == "b":
                        vt = mio.tile([P, B], f16, tag="vt", name="vt")
                        nc.vector.tensor_scalar(vt[:, cs], xt[:, cs], C(1, c),
                                                None, ALU.mult)
                        nc.vector.tensor_add(vt[:, cs], vt[:, cs], at[:, cs])
                        d["vt"] = vt
                        # squares in place (same queue, stays one stage)
                        nc.vector.tensor_mul(ut[:, cs], ut[:, cs], ut[:, cs])
                        nc.vector.tensor_mul(vt[:, cs], vt[:, cs], vt[:, cs])
                        xq = nc.gpsimd if xx_gp else nc.vector
                        xq.tensor_mul(xt[:, cs], xt[:, cs], xt[:, cs])
                    else:
                        vps = psum.tile([P, B], f32, tag="vps", name="vps")
                        d["vps"] = vps
                        for j in range(0, B, 512):
                            nc.tensor.matmul(vps[:, j:j + 512], DG(c),
                                             xt[:, j:j + 512],
                                             start=True, stop=False)
                        for j in range(0, B, 512):
                            nc.tensor.matmul(vps[:, j:j + 512], DG(nchunk),
                                             at[:, j:j + 512],
                                             start=False, stop=True)

                def s2(vc):
                    v_id, c = vc
                    d = st[v_id]
                    xt, ut = d["xt"], d["ut"]
                    if mode == "b":
                        return  # folded into s1 (same engine, in order)
                    nc.vector.tensor_mul(ut[:], ut[:], ut[:])   # uu
                    vt = mio.tile([P, B], f16, tag="vt", name="vt")
                    d["vt"] = vt
                    nc.scalar.activation(vt[:], d["vps"][:], AF.Square,
                                         scale=C(2, c))
                    nc.vector.tensor_mul(xt[:], xt[:], xt[:])   # xx

                def s3(vc):
                    v_id, c, lo, hi = vc
                    d = st[v_id]
                    if merged:
                        xuv = d["xuv"]
                        xt, ut, vt = xuv[:, 0, :], xuv[:, 1, :], xuv[:, 2, :]
                    else:
                        xt, ut, vt = d["xt"], d["ut"], d["vt"]
                    if mode == "b":
                        # W2 groups: diag(g2) @ vv  (+)  diag(c22) @ xx
                        # D2 groups: diag(q2) @ xx  (+)  I @ uu
                        groups = [(goff, min(HB, hi - goff))
                                  for goff in range(lo, hi, HB)]
                        wps = [psum.tile([P, HB], f32, tag="wp", name="wp")
                               for _ in groups]
                        dps = [psum.tile([P, HB], f32, tag="dp", name="dp")
                               for _ in groups]
                        d["wps"], d["dps"], d["groups"] = wps, dps, groups
                        mm = nc.tensor.matmul

                        def sweep(dst, dgb, src, start, stop):
                            for h, (goff, gw) in enumerate(groups):
                                for j in range(0, gw, 512):
                                    w = min(512, gw - j)
                                    mm(dst[h][:, j:j + w], dgb,
                                       src[:, goff + j:goff + j + w],
                                       start=start, stop=stop)

                        sweep(wps, DG(c), vt, True, False)
                        sweep(wps, DG(nchunk + c), xt, False, True)
                        sweep(dps, DG(2 * nchunk + c), xt, True, False)
                        sweep(dps, DG(3 * nchunk), ut, False, True)
                        # (vt/xt/ut are plain 2D views in merged mode too)
                    else:
                        w2 = mio.tile([P, B], f16, tag="w2", name="w2")
                        nc.vector.tensor_scalar(w2[:], xt[:], C(3, c), None,
                                                ALU.mult)
                        nc.vector.tensor_add(vt[:], vt[:], w2[:])
                        nc.vector.tensor_scalar(xt[:], xt[:], C(4, c), None,
                                                ALU.mult)
                        nc.vector.tensor_add(ut[:], ut[:], xt[:])

                def s4(vc):
                    v_id, c, lo, hi = vc
                    d = st.pop(v_id)
                    rs = slice(c * P, (c + 1) * P)
                    cs = slice(lo, hi)
                    if merged:
                        xuv = d["xuv"]
                        # sqrt outputs overwrite the xx / uu regions
                        vt, ut = xuv[:, 0, :], xuv[:, 1, :]
                    else:
                        ut, vt = d["ut"], d["vt"]
                    if mode == "b":
                        for h, (goff, gw) in enumerate(d["groups"]):
                            hs = slice(goff, goff + gw)
                            nc.scalar.activation(vt[:, hs],
                                                 d["wps"][h][:, 0:gw],
                                                 AF.Sqrt)
                            nc.scalar.activation(ut[:, hs],
                                                 d["dps"][h][:, 0:gw],
                                                 AF.Sqrt, scale=k2sq)
                    else:
                        nc.scalar.activation(vt[:], vt[:], AF.Sqrt,
                                             scale=1024.0)
                        nc.scalar.activation(ut[:], ut[:], AF.Sqrt,
                                             scale=k2sq)
                    getattr(nc, out_eng).dma_start(o1_ext[rs, cs], vt[:, cs])
                    o2q = nc.gpsimd if swdge_o2 else getattr(nc, out_eng)
                    o2q.dma_start(o2_ext[rs, cs], ut[:, cs])

                stages = ([s0, s1, s3, s4] if mode == "b"
                          else [s0, s1, s2, s3, s4])
                nstg = len(stages)
                # virtual chunk list: first/last row-chunks split into
                # column halves so the pipeline fills fast and drains with a
                # short tail; middle chunks full-width (lowest op overhead).
                # passes>1 repeats chunks to amortize fill/drain in probes.
                spans = []
                for c in range(nchunk):
                    if taper >= 2 and c == 0:
                        spans += [(c, 0, 512), (c, 512, 1024), (c, 1024, B)]
                    elif taper >= 2 and c == nchunk - 1:
                        spans += [(c, 0, 1024), (c, 1024, 1536),
                                  (c, 1536, B)]
                    elif taper == 1 and c in (0, nchunk - 1):
                        spans += [(c, 0, HB), (c, HB, B)]
                    else:
                        spans.append((c, 0, B))
                vchunks = [(p * len(spans) + i, c, lo, hi)
                           for p in range(passes)
                           for i, (c, lo, hi) in enumerate(spans)]
                nv = len(vchunks)
                for t in range(nv + nstg - 1):
                    for s in range(nstg - 1, -1, -1):
                        i = t - s
                        if 0 <= i < nv:
                            stages[s](vchunks[i])

            if loop_n > 1:
                with tc.For_i(0, loop_n, 1):
                    body()
            else:
                body()

    nc.compile()
    return nc


def _shard_inputs(input_signal, add_signal, coefs, s, vecs=None, mode=MODE):
    x = np.asarray(input_signal, dtype=np.float32).astype(np.float16)
    a = (np.asarray(add_signal, dtype=np.float32)
         * np.float32(s)).astype(np.float16)
    vecs = vecs or {}
    in_maps = []
    for i in range(NCORES):
        sl = slice(i * WSH, (i + 1) * WSH)
        # coef layout [P, NCOEF*NCHUNK]: column k*NCHUNK+c holds coef k of
        # chunk c; row p is wavelength c*P+p within the shard.
        cf = np.ascontiguousarray(
            coefs[:, sl].reshape(NCOEF, NCHUNK, P)
            .transpose(2, 0, 1).reshape(P, NCOEF * NCHUNK))
        m = {
            "x_t": np.ascontiguousarray(x[:, sl].T),
            "a_t": np.ascontiguousarray(a[:, sl].T),
            "cf_t": cf,
        }

        def diag_blocks(names):
            nd = len(names) * NCHUNK + 1
            dgm = np.zeros((P, nd * P), np.float16)
            for k, nm in enumerate(names):
                vsh = np.asarray(vecs[nm])[sl].reshape(NCHUNK, P)
                for c in range(NCHUNK):
                    blk = k * NCHUNK + c
                    dgm[:, blk * P:(blk + 1) * P] = np.diag(
                        vsh[c].astype(np.float16))
            dgm[:, (nd - 1) * P:] = np.eye(P, dtype=np.float16)
            return dgm

        if mode == "b":
            m["dg_t"] = diag_blocks(["g2", "c22", "q2"])
        else:
            m["dg_t"] = diag_blocks(["rp"])
        in_maps.append(m)
    return in_maps


def _gather_outputs(results):
    through = np.empty((B, W), np.float32)
    drop = np.empty((B, W), np.float32)
    for i in range(NCORES):
        sl = slice(i * WSH, (i + 1) * WSH)
        through[:, sl] = results[i]["o1_t"].T.astype(np.float32)
        drop[:, sl] = results[i]["o2_t"].T.astype(np.float32)
    return through, drop


def kernel(input_signal, add_signal, wavelengths, coupling_1, coupling_2,
           phi_1, phi_2, phi_ring, alpha):
    from concourse.bass_utils import run_bass_kernel_spmd

    coefs, sc = _host_prep(wavelengths, coupling_1, coupling_2, phi_1, phi_2,
                           phi_ring, alpha)
    nc = _build_graph(sc["k2c"])
    in_maps = _shard_inputs(input_signal, add_signal, coefs, sc["s"], vecs=sc)
    res = run_bass_kernel_spmd(nc, in_maps, core_ids=list(range(NCORES)))
    return _gather_outputs(res.results)



# revision 5
# speedup vs baseline: 1.1505x; 1.1505x over previous
"""Trainium2 Bass kernel for nn_AddDropMRR (add-drop microring resonator).

Math: both outputs are magnitudes of complex-linear maps of the two real
inputs, through = |alpha_w x + beta_w a|, drop = |ad_w x + bd_w a|, so

  through^2 = A x^2 + B xa + C a^2      A=|alpha|^2, B=2Re(alpha conj(beta)),
  drop^2    = D x^2 + E xa + F a^2      C=|beta|^2  (etc. for drop)

with all six coefficients per-wavelength functions of `wavelengths` and the
scalar params -> computed on HOST (complex128) and shipped once as f16
diagonal matmul blocks. A and C (D and F) are inflated by (1+2^-9) to keep
the quadratic form non-negative under f16 product rounding (the form is PSD:
discriminant = Im(alpha conj(beta))^2 >= 0), and the ACT sqrt adds a tiny
positive bias to absorb f32 accumulation dips.

Device graph per chunk (128 wavelengths x 2048 batch), software-pipelined:

  s0 DMA(sync q):  load x, a chunk                  (~3.2us/chunk shared-DMA)
  s1 DVE:          xa = x*a, xx = x*x (in place)    (~2.3us)
      Pool:        aa = a*a (in place)              (~1.7us)
  s2 PE:           PSUM_T[h] = dgA@xx + dgB@xa + dgC@aa   (per 1024-half,
                   PSUM_D[h] = dgD@xx + dgE@xa + dgF@aa    512-col groups)
  s3 ACT:          o1 = sqrt(PSUM_T + 1e-5), o2 = sqrt(PSUM_D + 1e-7) -> f16
                   o1 DMA on scalar q, o2 DMA on vector q

This cuts DVE busy from ~55us (baseline u/v rotation form: 7 DVE ops/chunk)
to ~18us (2 ops/chunk); PE does 6 diag sweeps/chunk (~46us) and the shared
DMA device (~50us for the 16MiB/core of fp16 I/O) becomes the bound.
Sharding: wavelength dim split 8 ways across cores (fully elementwise);
host transposes so wavelength lies on SBUF partitions. Coef tables load
outside the timing loop.
"""
import numpy as np

B = 2048           # batch
W = 8192           # wavelengths
NCORES = 8
WSH = W // NCORES  # 1024 wavelengths per core
P = 128            # SBUF partitions
NCHUNK = WSH // P  # 8 chunks per core
NCOEF = 6          # A, B, C, D, E, F
N_EFF = 2.4
CIRC = 2.0 * np.pi * 1e-05
DLT = 2.0 ** -9    # PSD inflation of A, C, D, F
MODE = "q"         # quadratic-form mode (only mode)
HB = B // 2        # psum half width


def _host_prep(wavelengths, coupling_1, coupling_2, phi_1, phi_2, phi_ring,
               alpha):
    """Six per-wavelength quadratic-form coefficients (complex128 host)."""
    c1 = float(np.asarray(coupling_1).reshape(-1)[0])
    c2 = float(np.asarray(coupling_2).reshape(-1)[0])
    p1 = float(np.asarray(phi_1).reshape(-1)[0])
    pr = float(np.asarray(phi_ring).reshape(-1)[0])
    al = float(np.asarray(alpha).reshape(-1)[0])
    k1c = float(np.clip(c1, 0.01, 0.99))
    k2c = float(np.clip(c2, 0.01, 0.99))
    t1 = float(np.sqrt(1.0 - k1c * k1c))
    t2 = float(np.sqrt(1.0 - k2c * k2c))
    s1 = float(np.sqrt(c1))      # unclamped, as in reference
    s = float(np.sqrt(c2))       # unclamped
    kappa = float(al * np.sqrt(1.0 - c1 * c1) * np.sqrt(1.0 - c2 * c2))

    # phi in f32 exactly as the reference computes it, then f64 trig
    wl = np.asarray(wavelengths, np.float32)
    phi32 = (np.float32(2.0 * np.pi * N_EFF) / wl) * np.float32(CIRC) \
        + np.float32(pr)
    phi = phi32.astype(np.float64)

    den = 1.0 - kappa * np.exp(1j * phi)
    ring = 1j * k1c * al * np.exp(1j * (phi + p1))     # ring one-pass factor
    alpha_t = t1 + t2 * s1 * ring / den                # through: x coef
    beta_t = (t2 * s1 * s) / den                       # through: a coef
    ad = k2c * ring                                    # |j e^{j phi2}| = 1
    bd = (k2c * s) * np.ones_like(phi)

    A = (np.abs(alpha_t) ** 2) * (1.0 + DLT)
    Bv = 2.0 * np.real(alpha_t * np.conj(beta_t))
    C = (np.abs(beta_t) ** 2) * (1.0 + DLT)
    D = (np.abs(ad) ** 2) * (1.0 + DLT)
    E = 2.0 * np.real(ad * np.conj(bd))
    F = (np.abs(bd) ** 2) * (1.0 + DLT)

    coefs = np.stack([A, Bv, C, D, E, F]).astype(np.float32)   # [NCOEF, W]
    return coefs, dict(s=s, k2c=k2c)


def _build_graph(k2c, loop_n=1, nchunk=NCHUNK, bufs=8, taper=1, **_ignored):
    """SPMD per-core graph; see module docstring. loop_n>1 wraps the body
    in an on-device For_i loop for steady-state timing; the coefficient
    table loads stay OUTSIDE the loop."""
    import concourse.tile as tile
    from concourse import bacc, mybir, bass

    f16 = mybir.dt.float16
    f32 = mybir.dt.float32
    AF = mybir.ActivationFunctionType
    ndiag = NCOEF * nchunk

    wsh = nchunk * P
    nc = bacc.Bacc("TRN2", target_bir_lowering=False, debug=False,
                   num_devices=NCORES)
    x_ext = nc.declare_dram_parameter("x_t", [wsh, B], f16, isOutput=False)
    a_ext = nc.declare_dram_parameter("a_t", [wsh, B], f16, isOutput=False)
    dg_ext = nc.declare_dram_parameter("dg_t", [P, ndiag * P], f16,
                                       isOutput=False)
    o1_ext = nc.declare_dram_parameter("o1_t", [wsh, B], f16, isOutput=True)
    o2_ext = nc.declare_dram_parameter("o2_t", [wsh, B], f16, isOutput=True)

    with tile.TileContext(nc) as tc:
        with tc.tile_pool(name="cst", bufs=1) as cst, \
             tc.tile_pool(name="mio", bufs=bufs) as mio, \
             tc.tile_pool(name="psum", bufs=2,
                          space=bass.MemorySpace.PSUM) as psum:

            dg = cst.tile([P, ndiag * P], f16, tag="dg", name="dg")
            nc.sync.dma_start(dg[:], dg_ext[:])
            b1 = cst.tile([P, 1], f32, tag="b1", name="b1")
            nc.vector.memset(b1[:], 1e-5)
            b2 = cst.tile([P, 1], f32, tag="b2", name="b2")
            nc.vector.memset(b2[:], 1e-7)

            def DG(k, c):  # diag block of coef k, chunk c
                b = k * nchunk + c
                return dg[:, b * P:(b + 1) * P]

            def body(_iv=None):
                st = {}

                def s0(vc):
                    v_id, c, lo, hi = vc
                    rs = slice(c * P, (c + 1) * P)
                    cs = slice(lo, hi)
                    xt = mio.tile([P, B], f16, tag="xt", name="xt")
                    nc.sync.dma_start(xt[:, cs], x_ext[rs, cs])
                    at = mio.tile([P, B], f16, tag="at", name="at")
                    nc.sync.dma_start(at[:, cs], a_ext[rs, cs])
                    st[v_id] = dict(xt=xt, at=at)

                def s1(vc):
                    v_id, c, lo, hi = vc
                    cs = slice(lo, hi)
                    d = st[v_id]
                    xt, at = d["xt"], d["at"]
                    xa = mio.tile([P, B], f16, tag="xa", name="xa")
                    nc.vector.tensor_mul(xa[:, cs], xt[:, cs], at[:, cs])
                    nc.vector.tensor_mul(xt[:, cs], xt[:, cs], xt[:, cs])
                    nc.gpsimd.tensor_mul(at[:, cs], at[:, cs], at[:, cs])
                    d["xa"] = xa

                def s2(vc):
                    v_id, c, lo, hi = vc
                    d = st[v_id]
                    xx, xa, aa = d["xt"], d["xa"], d["at"]
                    groups = [(goff, min(HB, hi - goff))
                              for goff in range(lo, hi, HB)]
                    tps = [psum.tile([P, HB], f32, tag="tp", name="tp")
                           for _ in groups]
                    dps = [psum.tile([P, HB], f32, tag="dp", name="dp")
                           for _ in groups]
                    d["tps"], d["dps"], d["groups"] = tps, dps, groups
                    mm = nc.tensor.matmul

                    def sweep(dst, dgb, src, start, stop):
                        for h, (goff, gw) in enumerate(groups):
                            for j in range(0, gw, 512):
                                w = min(512, gw - j)
                                mm(dst[h][:, j:j + w], dgb,
                                   src[:, goff + j:goff + j + w],
                                   start=start, stop=stop)

                    sweep(tps, DG(0, c), xx, True, False)
                    sweep(tps, DG(1, c), xa, False, False)
                    sweep(tps, DG(2, c), aa, False, True)
                    sweep(dps, DG(3, c), xx, True, False)
                    sweep(dps, DG(4, c), xa, False, False)
                    sweep(dps, DG(5, c), aa, False, True)

                def s3(vc):
                    v_id, c, lo, hi = vc
                    d = st.pop(v_id)
                    rs = slice(c * P, (c + 1) * P)
                    cs = slice(lo, hi)
                    o1t = mio.tile([P, B], f16, tag="o1t", name="o1t")
                    o2t = mio.tile([P, B], f16, tag="o2t", name="o2t")
                    for h, (goff, gw) in enumerate(d["groups"]):
                        hs = slice(goff, goff + gw)
                        nc.scalar.activation(o1t[:, hs], d["tps"][h][:, 0:gw],
                                             AF.Sqrt, bias=b1[:])
                        nc.scalar.activation(o2t[:, hs], d["dps"][h][:, 0:gw],
                                             AF.Sqrt, bias=b2[:])
                    nc.scalar.dma_start(o1_ext[rs, cs], o1t[:, cs])
                    nc.gpsimd.dma_start(o2_ext[rs, cs], o2t[:, cs])

                stages = [s0, s1, s2, s3]
                nstg = len(stages)
                # first/last chunks split into column halves so the pipeline
                # fills fast and drains with a short tail
                spans = []
                for c in range(nchunk):
                    if taper and c in (0, nchunk - 1):
                        spans += [(c, 0, HB), (c, HB, B)]
                    else:
                        spans.append((c, 0, B))
                vchunks = [(i, c, lo, hi)
                           for i, (c, lo, hi) in enumerate(spans)]
                nv = len(vchunks)
                for t in range(nv + nstg - 1):
                    for s in range(nstg - 1, -1, -1):
                        i = t - s
                        if 0 <= i < nv:
                            stages[s](vchunks[i])

            if loop_n > 1:
                with tc.For_i(0, loop_n, 1):
                    body()
            else:
                body()

    nc.compile()
    return nc


def _shard_inputs(input_signal, add_signal, coefs, s, vecs=None, mode=MODE):
    x = np.asarray(input_signal, dtype=np.float32).astype(np.float16)
    a = np.asarray(add_signal, dtype=np.float32).astype(np.float16)
    in_maps = []
    for i in range(NCORES):
        sl = slice(i * WSH, (i + 1) * WSH)
        dgm = np.zeros((P, NCOEF * NCHUNK * P), np.float16)
        csh = coefs[:, sl].reshape(NCOEF, NCHUNK, P)
        for k in range(NCOEF):
            for c in range(NCHUNK):
                b = k * NCHUNK + c
                dgm[:, b * P:(b + 1) * P] = np.diag(
                    csh[k, c].astype(np.float16))
        in_maps.append({
            "x_t": np.ascontiguousarray(x[:, sl].T),
            "a_t": np.ascontiguousarray(a[:, sl].T),
            "dg_t": dgm,
        })
    return in_maps


def _gather_outputs(results):
    through = np.empty((B, W), np.float32)
    drop = np.empty((B, W), np.float32)
    for i in range(NCORES):
        sl = slice(i * WSH, (i + 1) * WSH)
        through[:, sl] = results[i]["o1_t"].T.astype(np.float32)
        drop[:, sl] = results[i]["o2_t"].T.astype(np.float32)
    return through, drop


def kernel(input_signal, add_signal, wavelengths, coupling_1, coupling_2,
           phi_1, phi_2, phi_ring, alpha):
    from concourse.bass_utils import run_bass_kernel_spmd

    coefs, sc = _host_prep(wavelengths, coupling_1, coupling_2, phi_1, phi_2,
                           phi_ring, alpha)
    nc = _build_graph(sc["k2c"])
    in_maps = _shard_inputs(input_signal, add_signal, coefs, sc["s"])
    res = run_bass_kernel_spmd(nc, in_maps, core_ids=list(range(NCORES)))
    return _gather_outputs(res.results)
